# revision 33
# baseline (speedup 1.0000x reference)
"""AttentiveFP molecular readout kernel for 8x Trainium2 NeuronCores.

Data-parallel over the batch (128 molecules/core). Never materializes
xt = atom_FP @ attend_W.T: BatchNorm over xt is affine per channel, so
  sum_l attn*BN(xt) = (sum_l attn*atom) @ (diag(a)W).T + d
with a = rsqrt(var+eps)*gamma and d = a*(b - mean) + beta (sum_l attn
is exactly 1: masked scores underflow to exp(-9e8)=0).  Mean/var of xt
come from the Gram matrix atom^T@atom (one-time PE pass).  Exact BN
batch-stat parity across cores via small AllReduces (one early for the
Gram stats, one per layer for the score stats, plus a warm-up).

Atom data is packed host-side as bf16 [l, b, atom|1|mask] so one HBM
read serves the Gram pass, s2, super0 and every layer's weighted
reduction u = sum_l attn*atom (PE matmuls with the per-molecule atom
block stationary and a 1-column attn vector moving).  Atom DMA rides
the sync queue; all small/gather DMAs are batched onto the scalar
queue so the bulk stream starts immediately.
"""
import os
import sys

sys.path.insert(0, "/opt/trn_rl_repo")

from contextlib import ExitStack

import numpy as np
import ml_dtypes

import concourse.bacc as bacc
import concourse.bass as bass
import concourse.tile as tile
from concourse import masks, mybir
from concourse.bass_utils import run_bass_kernel_spmd

B, L, FP, LAYERS = 1024, 128, 256, 3
NCORES = 8
BLOC = B // NCORES          # 128 molecules per core
N = B * L                   # global BN sample count
EPS = 1e-6
NEG = -900000000.0
F32 = mybir.dt.float32
BF16 = mybir.dt.bfloat16
P = 128
FC = FP // P                # 2 f-chunks of 128
JC = 3 * FP // P            # 6 gate-row chunks of 128
RW = FP + 2                 # packed row: atom | 1.0 | mask
GBLK = 16                   # molecules per DMA chunk
NG = BLOC // GBLK
DEBUG_NO_COLLECTIVE = False
KSTAGE = int(os.environ.get('KSTAGE', '9'))
UTP = int(os.environ.get('UTP', '0'))   # column-tiled u-pass weight loads
ALU = mybir.AluOpType
ACT = mybir.ActivationFunctionType


def _bcast_ap(ap, parts=P):
    """Partition-broadcast view of a 1D/row AP (step-0 partition dim)."""
    return bass.AP(tensor=ap.tensor, offset=ap.offset, ap=[[0, parts]] + list(ap.ap))


def _gather_ap(handle, nchunk, rowlen):
    """Partition-major view of a [nchunk*128, rowlen]-ish flat DRAM tensor:
    dest (p, c) <- src[c*128 + p] (for rowlen==1 vectors) or
    dest (p, c, f) <- src[c*128 + p, f]."""
    base = handle[:]
    if rowlen == 1:
        ap = [[1, P], [P, nchunk]]
    else:
        ap = [[rowlen, P], [P * rowlen, nchunk], [1, rowlen]]
    return bass.AP(tensor=base.tensor, offset=base.offset, ap=ap)


def _build_kernel():
    nc = bacc.Bacc()
    atom_h = nc.declare_dram_parameter("atom_pk", [L, BLOC, RW], BF16, isOutput=False)
    mask_h = nc.declare_dram_parameter("atom_mask", [BLOC, L], F32, isOutput=False)
    alw_h = nc.declare_dram_parameter("align_w", [1, 2 * FP], F32, isOutput=False)
    alb_h = nc.declare_dram_parameter("align_b", [1], F32, isOutput=False)
    alg_h = nc.declare_dram_parameter("align_gamma", [1], F32, isOutput=False)
    albe_h = nc.declare_dram_parameter("align_beta", [1], F32, isOutput=False)
    atw_h = nc.declare_dram_parameter("attend_w", [FP, FP], F32, isOutput=False)
    atb_h = nc.declare_dram_parameter("attend_b", [FP], F32, isOutput=False)
    atg_h = nc.declare_dram_parameter("attend_gamma", [FP], F32, isOutput=False)
    atbe_h = nc.declare_dram_parameter("attend_beta", [FP], F32, isOutput=False)
    wih_h = nc.declare_dram_parameter("gru_wih", [3 * FP, FP], F32, isOutput=False)
    whh_h = nc.declare_dram_parameter("gru_whh", [3 * FP, FP], F32, isOutput=False)
    bih_h = nc.declare_dram_parameter("gru_bih", [3 * FP], F32, isOutput=False)
    bhh_h = nc.declare_dram_parameter("gru_bhh", [3 * FP], F32, isOutput=False)
    osuper_h = nc.declare_dram_parameter("out_super", [BLOC, FP], F32, isOutput=True)
    oact_h = nc.declare_dram_parameter("out_act", [BLOC, FP], F32, isOutput=True)

    with tile.TileContext(nc) as tc, ExitStack() as ctx:
        _body(ctx, tc, atom_h, mask_h, alw_h, alb_h, alg_h, albe_h,
              atw_h, atb_h, atg_h, atbe_h, wih_h, whh_h, bih_h, bhh_h,
              osuper_h, oact_h)
    nc.finalize()
    return nc


def _body(ctx, tc, atom_h, mask_h, alw_h, alb_h, alg_h, albe_h,
          atw_h, atb_h, atg_h, atbe_h, wih_h, whh_h, bih_h, bhh_h,
          osuper_h, oact_h):
    nc = tc.nc
    singles = ctx.enter_context(tc.tile_pool(name="singles", bufs=1))
    small = ctx.enter_context(tc.tile_pool(name="small", bufs=2))
    med = ctx.enter_context(tc.tile_pool(name="med", bufs=2))
    s2p = ctx.enter_context(tc.tile_pool(name="s2p", bufs=2))
    dram = ctx.enter_context(tc.tile_pool(name="dram", bufs=1, space="DRAM"))
    ps_t = ctx.enter_context(tc.tile_pool(name="ps_t", bufs=2, space="PSUM"))
    ps_mm = ctx.enter_context(tc.tile_pool(name="ps_mm", bufs=1, space="PSUM"))

    # ---- atom bulk first on the sync DMA queue --------------------------
    atile = [singles.tile([P, GBLK, RW], BF16, name=f"at{g}", tag=f"at{g}")
             for g in range(NG)]
    for g in range(NG):
        nc.sync.dma_start(out=atile[g][:], in_=atom_h[:, g * GBLK:(g + 1) * GBLK, :])

    # ---- batched prep DMAs on the scalar queue --------------------------
    w_sb = singles.tile([P, FC, FP], F32)
    nc.scalar.dma_start(out=w_sb[:], in_=_gather_ap(atw_h, FC, FP))
    gru_ih = singles.tile([P, JC, FP], F32)
    gru_hh = singles.tile([P, JC, FP], F32)
    nc.scalar.dma_start(out=gru_ih[:], in_=_gather_ap(wih_h, JC, FP))
    nc.scalar.dma_start(out=gru_hh[:], in_=_gather_ap(whh_h, JC, FP))
    w1_pm = singles.tile([P, FC], F32)
    nc.scalar.dma_start(out=w1_pm[:], in_=_gather_ap(alw_h, FC, 1))
    atb_pm = singles.tile([P, FC], F32)
    atg_pm = singles.tile([P, FC], F32)
    atbe_pm = singles.tile([P, FC], F32)
    nc.scalar.dma_start(out=atb_pm[:], in_=_gather_ap(atb_h, FC, 1))
    nc.scalar.dma_start(out=atg_pm[:], in_=_gather_ap(atg_h, FC, 1))
    nc.scalar.dma_start(out=atbe_pm[:], in_=_gather_ap(atbe_h, FC, 1))
    bih_pm = singles.tile([P, JC], F32)
    bhh_pm = singles.tile([P, JC], F32)
    nc.scalar.dma_start(out=bih_pm[:], in_=_gather_ap(bih_h, JC, 1))
    nc.scalar.dma_start(out=bhh_pm[:], in_=_gather_ap(bhh_h, JC, 1))
    w2_bc = singles.tile([P, FP], F32)
    nc.scalar.dma_start(out=w2_bc[:], in_=_bcast_ap(alw_h[0, FP:2 * FP]))
    al_bc = singles.tile([P, 3], F32)   # [align_b, align_gamma, align_beta]
    nc.scalar.dma_start(out=al_bc[:, 0], in_=_bcast_ap(alb_h[0:1])[:, 0])
    nc.scalar.dma_start(out=al_bc[:, 1], in_=_bcast_ap(alg_h[0:1])[:, 0])
    nc.scalar.dma_start(out=al_bc[:, 2], in_=_bcast_ap(albe_h[0:1])[:, 0])
    mask_sb = singles.tile([P, L], F32)
    nc.scalar.dma_start(out=mask_sb[:], in_=mask_h[:, :])

    # ---- constants ------------------------------------------------------
    ident = singles.tile([P, P], F32)
    masks.make_identity(nc, ident[:])
    ident_bf = singles.tile([P, P], BF16)
    nc.vector.tensor_copy(ident_bf[:], ident[:])
    zero_pm = singles.tile([P, 1], F32)
    nc.vector.memset(zero_pm[:], 0.0)
    eps_pm = singles.tile([P, 1], F32)
    nc.vector.memset(eps_pm[:], EPS)
    ones_pm = singles.tile([P, 1], F32)
    nc.vector.memset(ones_pm[:], 1.0)

    def pe_t(out_sb, in_sb):
        """128x128 fp32 transpose via PE; out_sb <- in_sb.T"""
        pt = ps_t.tile([P, P], F32, name="ps_t", tag="ps_t")
        nc.tensor.transpose(pt[:], in_sb, ident[:])
        nc.vector.tensor_copy(out_sb, pt[:])

    # w2 replicated across the packed-chunk layout (zeros on ones/mask cols)
    w2b_bf = singles.tile([P, FP], BF16)
    nc.vector.tensor_copy(w2b_bf[:], w2_bc[:])
    w2rep = singles.tile([P, GBLK, RW], BF16)
    nc.vector.memset(w2rep[:], 0.0)
    for j in range(GBLK):
        nc.vector.tensor_copy(w2rep[:, j, 0:FP], w2b_bf[:])

    # attend-W transposes (needed early for dgw); GRU transposes deferred
    wt_sb = [singles.tile([P, FP], F32, name=f"wt_sb{i}", tag=f"wt_sb{i}") for i in range(FC)]
    for ic in range(FC):
        for fc in range(FC):
            pe_t(wt_sb[ic][:, fc * P:(fc + 1) * P],
                 w_sb[:, fc, ic * P:(ic + 1) * P])
    negm_sb = singles.tile([P, L], F32)
    nc.vector.tensor_scalar(out=negm_sb[:], in0=mask_sb[:], scalar1=1.0,
                            scalar2=-NEG, op0=ALU.subtract, op1=ALU.mult)
    bsum_pm = singles.tile([P, 4], F32)
    nc.vector.tensor_tensor(out=bsum_pm[:], in0=bih_pm[:, 0:4], in1=bhh_pm[:, 0:4],
                            op=ALU.add)

    if KSTAGE <= 0:
        dm = med.tile([P, FP], F32, name="dm", tag="dm", bufs=1)
        nc.vector.tensor_copy(dm[:], w_sb[:, 0, :])
        nc.sync.dma_start(out=osuper_h[:, :], in_=dm[:])
        nc.sync.dma_start(out=oact_h[:, :], in_=gru_ih[:, 0, :])
        return

    # ---------------- pass 1: Gram + supt (PE), s2 (DVE) ----------------
    gram_pool = tc.tile_pool(name="ps_gram", bufs=1, space="PSUM")
    ps_gram = gram_pool.__enter__()
    gram_ps = [ps_gram.tile([P, FP + 1], F32, name=f"gram{i}", tag=f"gram{i}") for i in range(FC)]
    supt_ps = ps_gram.tile([P, FC, P], F32, name="supt_ps", tag="supt_ps")
    s2t_sb = singles.tile([P, BLOC], F32)
    for g in range(NG):
        at = atile[g]
        for j in range(GBLK):
            b = g * GBLK + j
            for fc in range(FC):
                wsl = at[:, j, fc * P:(fc + 1) * P]
                nc.tensor.matmul(gram_ps[fc][:], wsl, at[:, j, 0:FP + 1],
                                 start=(b == 0), stop=(b == BLOC - 1))
                nc.tensor.matmul(supt_ps[:, fc, b:b + 1], wsl,
                                 at[:, j, FP + 1:FP + 2], start=True, stop=True)
        # s2 for this chunk: one dense product then per-molecule reduce.
        # The last 3 chunks are deferred past the AllReduce-1a trigger so
        # the Gram copies/dgw don't queue behind them on the DVE.
        if g < NG - 3:
            prod = s2p.tile([P, GBLK, RW], BF16, name="s2prod", tag="s2prod")
            nc.vector.tensor_tensor(out=prod[:], in0=at[:], in1=w2rep[:], op=ALU.mult)
            nc.vector.tensor_reduce(out=s2t_sb[:, g * GBLK:(g + 1) * GBLK],
                                    in_=prod[:], axis=mybir.AxisListType.X, op=ALU.add)

    g_sb = [singles.tile([P, FP + 1], F32, name=f"g_sb{i}", tag=f"g_sb{i}") for i in range(FC)]
    supt = [med.tile([P, P], F32, name=f"supt{i}", tag=f"supt{i}") for i in range(FC)]
    for fc in range(FC):
        nc.vector.tensor_copy(g_sb[fc][:], gram_ps[fc][:])
        nc.vector.tensor_copy(supt[fc][:], supt_ps[:, fc, :])
    gram_pool.__exit__(None, None, None)
    ps_g = ctx.enter_context(tc.tile_pool(name="ps_g", bufs=2, space="PSUM"))
    ps_u = ctx.enter_context(tc.tile_pool(name="ps_u", bufs=1, space="PSUM"))
    s2_sb = singles.tile([P, L], F32)

    def emit_outputs(supt_tiles):
        super_sb = med.tile([P, FP], F32, name="souts", tag="souts", bufs=1)
        for ic in range(FC):
            pe_t(super_sb[:, ic * P:(ic + 1) * P], supt_tiles[ic][:])
        act_sb = med.tile([P, FP], F32, name="souta", tag="souta", bufs=1)
        nc.scalar.activation(act_sb[:], super_sb[:], ACT.Relu, bias=zero_pm[:])
        nc.sync.dma_start(out=osuper_h[:, :], in_=super_sb[:])
        nc.sync.dma_start(out=oact_h[:, :], in_=act_sb[:])

    if KSTAGE <= 1:
        emit_outputs(supt)
        return

    # diag(W G W^T) per f-chunk -> [128,1]
    tt_scr = small.tile([P, FP], F32, name="ttscr", tag="ttscr")
    dgw_pm = [small.tile([P, 1], F32, name=f"dgw{i}", tag=f"dgw{i}") for i in range(FC)]
    for fc in range(FC):
        t1 = ps_mm.tile([P, FP], F32, name="t1", tag="mm")
        for ic in range(FC):
            nc.tensor.matmul(t1[:], wt_sb[ic][:, fc * P:(fc + 1) * P],
                             g_sb[ic][:, 0:FP],
                             start=(ic == 0), stop=(ic == FC - 1))
        nc.vector.tensor_tensor(out=tt_scr[:], in0=t1[:], in1=w_sb[:, fc, :],
                                op=ALU.mult)
        nc.vector.tensor_reduce(out=dgw_pm[fc][:], in_=tt_scr[:],
                                axis=mybir.AxisListType.X, op=ALU.add)

    # ---------------- AllReduce #1a: Gram-derived stats -----------------
    st_in = dram.tile([1, 2 * FP], F32, name="st_in", tag="st_in")
    st_out = dram.tile([1, 2 * FP], F32, name="st_out", tag="st_out")
    for fc in range(FC):
        nc.sync.dma_start(out=st_in[0, fc * P:(fc + 1) * P], in_=dgw_pm[fc][:, 0])
        nc.sync.dma_start(out=st_in[0, FP + fc * P:FP + (fc + 1) * P],
                          in_=g_sb[fc][:, FP])
    if DEBUG_NO_COLLECTIVE:
        nc.sync.dma_start(out=st_out[:], in_=st_in[:])
    else:
        nc.gpsimd.collective_compute(
            "AllReduce", ALU.add, replica_groups=[list(range(NCORES))],
            ins=[st_in[:].opt()], outs=[st_out[:].opt()])
    dgw_g = [small.tile([P, 1], F32, name=f"dgwg{i}", tag=f"dgwg{i}") for i in range(FC)]
    cs_g = [small.tile([P, 1], F32, name=f"csg{i}", tag=f"csg{i}") for i in range(FC)]
    for fc in range(FC):
        nc.sync.dma_start(out=dgw_g[fc][:, 0], in_=st_out[0, fc * P:(fc + 1) * P])
        nc.sync.dma_start(out=cs_g[fc][:, 0],
                          in_=st_out[0, FP + fc * P:FP + (fc + 1) * P])

    # GRU weight transposes: fill the PE queue while AllReduce 1a flies
    wiht = [singles.tile([P, 3 * FP], F32, name=f"wiht{i}", tag=f"wiht{i}") for i in range(FC)]
    whht = [singles.tile([P, 3 * FP], F32, name=f"whht{i}", tag=f"whht{i}") for i in range(FC)]
    for src, dst in ((gru_ih, wiht), (gru_hh, whht)):
        for jc in range(JC):
            for ic in range(FC):
                pe_t(dst[ic][:, jc * P:(jc + 1) * P],
                     src[:, jc, ic * P:(ic + 1) * P])

    # deferred s2 chunks run while AllReduce #1a is in flight
    for g in range(NG - 3, NG):
        prod = s2p.tile([P, GBLK, RW], BF16, name="s2prod", tag="s2prod")
        nc.vector.tensor_tensor(out=prod[:], in0=atile[g][:], in1=w2rep[:], op=ALU.mult)
        nc.vector.tensor_reduce(out=s2t_sb[:, g * GBLK:(g + 1) * GBLK],
                                in_=prod[:], axis=mybir.AxisListType.X, op=ALU.add)
    pe_t(s2_sb[:], s2t_sb[:])

    # ---------------- layer-0 state + score stats -----------------------
    actt = [med.tile([P, P], F32, name=f"actt{i}", tag=f"actt{i}") for i in range(FC)]
    for ic in range(FC):
        nc.scalar.activation(actt[ic][:], supt[ic][:], ACT.Relu, bias=zero_pm[:])

    def score_stats(actt_tiles, ss2_sb, score_sb):
        """score = s1 + s2 + align_b; writes local sum/sumsq scalars [1,2]."""
        s1_ps = ps_mm.tile([P, 1], F32, name="s1", tag="mm")
        for ic in range(FC):
            nc.tensor.matmul(s1_ps[:], actt_tiles[ic][:], w1_pm[:, ic:ic + 1],
                             start=(ic == 0), stop=(ic == FC - 1))
        s1_sb = small.tile([P, 1], F32, name="s1sb", tag="s1sb")
        nc.vector.tensor_copy(s1_sb[:], s1_ps[:])
        nc.vector.tensor_scalar(out=score_sb, in0=s2_sb[:], scalar1=s1_sb[:],
                                scalar2=al_bc[:, 0:1], op0=ALU.add, op1=ALU.add)
        red2 = small.tile([P, 2], F32, name="red2", tag="red2")
        nc.vector.tensor_reduce(out=red2[:, 0:1], in_=score_sb,
                                axis=mybir.AxisListType.X, op=ALU.add)
        scr = small.tile([P, L], F32, name="sqscr", tag="sqscr")
        nc.vector.tensor_tensor(out=scr[:], in0=score_sb, in1=score_sb,
                                op=ALU.mult)
        nc.vector.tensor_reduce(out=red2[:, 1:2], in_=scr[:],
                                axis=mybir.AxisListType.X, op=ALU.add)
        pred = ps_mm.tile([1, 2], F32, name="pred", tag="mm")
        nc.tensor.matmul(pred[:], ones_pm[:], red2[:], start=True, stop=True)
        nc.vector.tensor_copy(ss2_sb, pred[:])

    score_sb0 = med.tile([P, L], F32, name="score", tag="score")
    ss_sb = small.tile([1, 2], F32, name="ss0", tag="ss0")
    score_stats(actt, ss_sb[0:1, 0:2], score_sb0[:])

    # ---------------- AllReduce #1b: layer-0 score stats ----------------
    st1b_in = dram.tile([1, 2], F32, name="st1bi", tag="st1bi")
    st1b_out = dram.tile([1, 2], F32, name="st1bo", tag="st1bo")
    nc.sync.dma_start(out=st1b_in[0, 0:2], in_=ss_sb[0, 0:2])
    if DEBUG_NO_COLLECTIVE:
        nc.sync.dma_start(out=st1b_out[:], in_=st1b_in[:])
    else:
        nc.gpsimd.collective_compute(
            "AllReduce", ALU.add, replica_groups=[list(range(NCORES))],
            ins=[st1b_in[:].opt()], outs=[st1b_out[:].opt()])
    sums_bc = small.tile([P, 2], F32, name="sumsbc", tag="sumsbc")
    nc.sync.dma_start(out=sums_bc[:], in_=_bcast_ap(st1b_out[0, 0:2]))

    if KSTAGE <= 2:
        emit_outputs(supt)
        return

    # xt-BN affine: a = rsqrt(v+eps)*gamma, d = a*(b-m)+beta  (partition-major)
    a_pm = [small.tile([P, 1], F32, name=f"apm{i}", tag=f"apm{i}") for i in range(FC)]
    d_pm = [small.tile([P, 1], F32, name=f"dpm{i}", tag=f"dpm{i}") for i in range(FC)]
    adr = dram.tile([1, 2 * FP], F32, name="adr", tag="adr")
    for fc in range(FC):
        wc = ps_mm.tile([P, 1], F32, name="wc", tag="mm")
        for ic in range(FC):
            nc.tensor.matmul(wc[:], wt_sb[ic][:, fc * P:(fc + 1) * P], cs_g[ic][:],
                             start=(ic == 0), stop=(ic == FC - 1))
        m_xt = small.tile([P, 1], F32, name="mxt", tag="mxt")
        nc.vector.tensor_scalar(out=m_xt[:], in0=wc[:], scalar1=1.0 / N,
                                scalar2=atb_pm[:, fc:fc + 1], op0=ALU.mult, op1=ALU.add)
        e2 = small.tile([P, 1], F32, name="e2", tag="e2")
        # e2 = dgw/N + 2*b*wc/N + b^2
        nc.vector.tensor_scalar(out=e2[:], in0=wc[:], scalar1=2.0 / N,
                                scalar2=atb_pm[:, fc:fc + 1], op0=ALU.mult, op1=ALU.mult)
        nc.vector.tensor_scalar(out=tt_scr[:, 0:1], in0=dgw_g[fc][:],
                                scalar1=1.0 / N, scalar2=None, op0=ALU.mult)
        nc.vector.tensor_add(out=e2[:], in0=e2[:], in1=tt_scr[:, 0:1])
        nc.vector.tensor_tensor(out=tt_scr[:, 1:2], in0=atb_pm[:, fc:fc + 1],
                                in1=atb_pm[:, fc:fc + 1], op=ALU.mult)
        nc.vector.tensor_add(out=e2[:], in0=e2[:], in1=tt_scr[:, 1:2])
        # v = e2 - m^2 ; a = gamma/sqrt(v+eps)
        nc.vector.tensor_tensor(out=tt_scr[:, 2:3], in0=m_xt[:], in1=m_xt[:],
                                op=ALU.mult)
        nc.vector.tensor_sub(out=e2[:], in0=e2[:], in1=tt_scr[:, 2:3])
        nc.scalar.activation(e2[:], e2[:], ACT.Sqrt, bias=eps_pm[:])
        nc.vector.reciprocal(e2[:], e2[:])
        nc.vector.tensor_tensor(out=a_pm[fc][:], in0=e2[:], in1=atg_pm[:, fc:fc + 1],
                                op=ALU.mult)
        # d = a*(b - m) + beta
        nc.vector.tensor_sub(out=tt_scr[:, 3:4], in0=atb_pm[:, fc:fc + 1], in1=m_xt[:])
        nc.vector.tensor_tensor(out=tt_scr[:, 3:4], in0=tt_scr[:, 3:4],
                                in1=a_pm[fc][:], op=ALU.mult)
        nc.vector.tensor_scalar(out=d_pm[fc][:], in0=tt_scr[:, 3:4], scalar1=0.0,
                                scalar2=atbe_pm[:, fc:fc + 1], op0=ALU.add, op1=ALU.add)
        nc.sync.dma_start(out=adr[0, fc * P:(fc + 1) * P], in_=a_pm[fc][:, 0])
        nc.sync.dma_start(out=adr[0, FP + fc * P:FP + (fc + 1) * P],
                          in_=d_pm[fc][:, 0])
    a_bc = singles.tile([P, FP], F32)
    d_bc = singles.tile([P, FP], F32)
    nc.sync.dma_start(out=a_bc[:], in_=_bcast_ap(adr[0, 0:FP]))
    nc.sync.dma_start(out=d_bc[:], in_=_bcast_ap(adr[0, FP:2 * FP]))
    wpt = [singles.tile([P, FP], F32, name=f"wpt{i}", tag=f"wpt{i}") for i in range(FC)]
    for ic in range(FC):
        nc.vector.tensor_tensor(out=wpt[ic][:], in0=wt_sb[ic][:], in1=a_bc[:],
                                op=ALU.mult)

    def bn_coeffs(sums_bc_ap, scale_sb, shift_sb):
        """score-BN: scale = gamma*rstd, shift = beta - mean*scale (per-part)."""
        mean = small.tile([P, 1], F32, name="bnm", tag="bnm")
        nc.vector.tensor_scalar(out=mean[:], in0=sums_bc_ap[:, 0:1],
                                scalar1=1.0 / N, scalar2=None, op0=ALU.mult)
        var = small.tile([P, 1], F32, name="bnv", tag="bnv")
        nc.vector.tensor_scalar(out=var[:], in0=sums_bc_ap[:, 1:2],
                                scalar1=1.0 / N, scalar2=None, op0=ALU.mult)
        nc.vector.tensor_tensor(out=scale_sb, in0=mean[:], in1=mean[:],
                                op=ALU.mult)
        nc.vector.tensor_sub(out=var[:], in0=var[:], in1=scale_sb)
        nc.scalar.activation(var[:], var[:], ACT.Sqrt, bias=eps_pm[:])
        nc.vector.reciprocal(var[:], var[:])
        nc.vector.tensor_tensor(out=scale_sb, in0=var[:], in1=al_bc[:, 1:2],
                                op=ALU.mult)
        nc.vector.tensor_tensor(out=shift_sb, in0=mean[:], in1=scale_sb,
                                op=ALU.mult)
        nc.vector.tensor_scalar(out=shift_sb, in0=shift_sb, scalar1=-1.0,
                                scalar2=al_bc[:, 2:3], op0=ALU.mult, op1=ALU.add)

    sc_scale = small.tile([P, 1], F32, name="scsc", tag="scsc")
    sc_shift = small.tile([P, 1], F32, name="scsh", tag="scsh")
    bn_coeffs(sums_bc, sc_scale[:], sc_shift[:])

    if KSTAGE <= 3:
        emit_outputs(supt)
        return

    # ---------------- layers ----------------
    score_sb = score_sb0
    st2 = [(dram.tile([1, 2], F32, name=f"st2i{k}", tag=f"st2i{k}"),
            dram.tile([1, 2], F32, name=f"st2o{k}", tag=f"st2o{k}")) for k in range(2)]

    nlayers = 1 if KSTAGE <= 4 else LAYERS
    for k in range(nlayers):
        # BN + leaky-relu (fused) + additive mask
        y_sb = med.tile([P, L], F32, name="y", tag="y")
        nc.vector.tensor_scalar(out=y_sb[:], in0=score_sb[:], scalar1=sc_scale[:],
                                scalar2=sc_shift[:], op0=ALU.mult, op1=ALU.add)
        lr_r = med.tile([P, L], F32, name="lr_r", tag="lr_r")
        nc.scalar.activation(lr_r[:], y_sb[:], ACT.Relu, bias=zero_pm[:])
        nc.vector.tensor_sub(out=y_sb[:], in0=y_sb[:], in1=lr_r[:])
        nc.vector.tensor_scalar(out=y_sb[:], in0=y_sb[:], scalar1=0.01,
                                scalar2=None, op0=ALU.mult)
        nc.vector.tensor_add(out=y_sb[:], in0=y_sb[:], in1=lr_r[:])
        nc.vector.tensor_add(out=y_sb[:], in0=y_sb[:], in1=negm_sb[:])
        # softmax over l
        rmax = small.tile([P, 1], F32, name="rmax", tag="rmax")
        nc.vector.tensor_reduce(out=rmax[:], in_=y_sb[:],
                                axis=mybir.AxisListType.X, op=ALU.max,
                                negate=True)
        e_sb = med.tile([P, L], F32, name="esb", tag="esb")
        den = small.tile([P, 1], F32, name="den", tag="den")
        nc.vector.tensor_scalar(out=e_sb[:], in0=y_sb[:], scalar1=rmax[:],
                                scalar2=-80.0, op0=ALU.add, op1=ALU.max)
        nc.scalar.activation(e_sb[:], e_sb[:], ACT.Exp, bias=zero_pm[:])
        nc.vector.tensor_reduce(out=den[:], in_=e_sb[:],
                                axis=mybir.AxisListType.X, op=ALU.add)
        nc.vector.reciprocal(den[:], den[:])
        # attn (bf16, transposed to [l, b]); mask applied via packed mask col
        at_bf = med.tile([P, L], BF16, name="atbf", tag="atbf")
        nc.vector.tensor_scalar(out=at_bf[:], in0=e_sb[:], scalar1=den[:],
                                scalar2=None, op0=ALU.mult)
        att_ps = ps_t.tile([P, P], BF16, name="attps", tag="ps_t")
        nc.tensor.matmul(att_ps[:], at_bf[:], ident_bf[:], is_transpose=True)
        attnT = med.tile([P, P], BF16, name="attnT", tag="attnT")
        for g in range(NG):
            nc.vector.tensor_tensor(
                out=attnT[:, g * GBLK:(g + 1) * GBLK],
                in0=att_ps[:, g * GBLK:(g + 1) * GBLK],
                in1=atile[g][:, :, FP + 1], op=ALU.mult)

        # u^T = sum_l attn*atom via PE: atom block stationary, attn col moving
        u_ps = ps_u.tile([P, FC, P], F32, name="u_ps", tag="u_ps")
        for g in range(NG):
            at = atile[g]
            for j in range(GBLK):
                b = g * GBLK + j
                for fc in range(FC):
                    if UTP:
                        for t in range(4):
                            c0 = fc * P + 32 * t
                            nc.tensor.matmul(
                                u_ps[32 * t:32 * (t + 1), fc, b:b + 1],
                                at[:, j, c0:c0 + 32], attnT[:, b:b + 1],
                                start=True, stop=True, tile_position=(0, 32 * t))
                    else:
                        nc.tensor.matmul(u_ps[:, fc, b:b + 1],
                                         at[:, j, fc * P:(fc + 1) * P],
                                         attnT[:, b:b + 1], start=True, stop=True)
        u_sb = [med.tile([P, P], F32, name=f"usb{i}", tag=f"usb{i}") for i in range(FC)]
        for fc in range(FC):
            nc.vector.tensor_copy(u_sb[fc][:], u_ps[:, fc, :])

        # ctx = elu(u @ Wp.T + d)   (sum_l attn == 1 exactly)
        cpre_ps = ps_mm.tile([P, FP], F32, name="cpre", tag="mm")
        for ic in range(FC):
            nc.tensor.matmul(cpre_ps[:], u_sb[ic][:], wpt[ic][:],
                             start=(ic == 0), stop=(ic == FC - 1))
        cpre = med.tile([P, FP], F32, name="cprs", tag="cprs", bufs=1)
        nc.vector.tensor_add(out=cpre[:], in0=cpre_ps[:], in1=d_bc[:])
        relu_p = med.tile([P, FP], F32, name="relup", tag="relup", bufs=1)
        nc.scalar.activation(relu_p[:], cpre[:], ACT.Relu, bias=zero_pm[:])
        nc.vector.tensor_sub(out=cpre[:], in0=cpre[:], in1=relu_p[:])
        nc.scalar.activation(cpre[:], cpre[:], ACT.Exp, bias=zero_pm[:])
        ctx_sb = med.tile([P, FP], F32, name="ctx", tag="ctx", bufs=1)
        nc.vector.tensor_scalar(out=ctx_sb[:], in0=cpre[:], scalar1=1.0,
                                scalar2=None, op0=ALU.subtract)
        nc.vector.tensor_add(out=ctx_sb[:], in0=ctx_sb[:], in1=relu_p[:])

        # GRU (transposed form)
        ctxt = [med.tile([P, P], F32, name=f"ctxt{i}", tag=f"ctxt{i}") for i in range(FC)]
        for ic in range(FC):
            pe_t(ctxt[ic][:], ctx_sb[:, ic * P:(ic + 1) * P])
        new_supt = [med.tile([P, P], F32, name=f"nsupt{i}", tag=f"nsupt{i}")
                    for i in range(FC)]
        for c in range(FC):
            t = med.tile([P, P], F32, name="gt", tag="gt")
            r = med.tile([P, P], F32, name="gr", tag="gr")
            z = med.tile([P, P], F32, name="gz", tag="gz")
            nn_ = med.tile([P, P], F32, name="gn", tag="gn")
            grz = ps_g.tile([P, P], F32, name="grz", tag="grz")
            for ic in range(FC):
                nc.tensor.matmul(grz[:], wiht[ic][:, c * P:(c + 1) * P],
                                 ctxt[ic][:], start=(ic == 0), stop=False)
            for ic in range(FC):
                nc.tensor.matmul(grz[:], whht[ic][:, c * P:(c + 1) * P],
                                 supt[ic][:], start=False, stop=(ic == FC - 1))
            nc.scalar.activation(r[:], grz[:], ACT.Sigmoid, bias=bsum_pm[:, c:c + 1])
            grz2 = ps_g.tile([P, P], F32, name="grz2", tag="grz")
            jz = 2 + c
            for ic in range(FC):
                nc.tensor.matmul(grz2[:], wiht[ic][:, jz * P:(jz + 1) * P],
                                 ctxt[ic][:], start=(ic == 0), stop=False)
            for ic in range(FC):
                nc.tensor.matmul(grz2[:], whht[ic][:, jz * P:(jz + 1) * P],
                                 supt[ic][:], start=False, stop=(ic == FC - 1))
            nc.scalar.activation(z[:], grz2[:], ACT.Sigmoid, bias=bsum_pm[:, jz:jz + 1])
            jn = 4 + c
            gin = ps_g.tile([P, P], F32, name="gin", tag="gin", bufs=1)
            ghn = ps_g.tile([P, P], F32, name="ghn", tag="ghn", bufs=1)
            for ic in range(FC):
                nc.tensor.matmul(gin[:], wiht[ic][:, jn * P:(jn + 1) * P],
                                 ctxt[ic][:], start=(ic == 0), stop=(ic == FC - 1))
                nc.tensor.matmul(ghn[:], whht[ic][:, jn * P:(jn + 1) * P],
                                 supt[ic][:], start=(ic == 0), stop=(ic == FC - 1))
            nc.vector.tensor_scalar(out=t[:], in0=ghn[:],
                                    scalar1=bhh_pm[:, jn:jn + 1], scalar2=None,
                                    op0=ALU.add)
            nc.vector.tensor_tensor(out=t[:], in0=t[:], in1=r[:], op=ALU.mult)
            nc.vector.tensor_add(out=t[:], in0=t[:], in1=gin[:])
            nc.scalar.activation(nn_[:], t[:], ACT.Tanh, bias=bih_pm[:, jn:jn + 1])
            # super = n + z*(super - n)
            nc.vector.tensor_sub(out=t[:], in0=supt[c][:], in1=nn_[:])
            nc.vector.tensor_tensor(out=t[:], in0=t[:], in1=z[:], op=ALU.mult)
            nc.vector.tensor_add(out=new_supt[c][:], in0=nn_[:], in1=t[:])
        supt = new_supt

        if KSTAGE <= 4 and k == 0:
            emit_outputs(supt)
            return
        if k < nlayers - 1:
            actt = [med.tile([P, P], F32, name=f"actt{i}", tag=f"actt{i}") for i in range(FC)]
            for ic in range(FC):
                nc.scalar.activation(actt[ic][:], supt[ic][:], ACT.Relu, bias=zero_pm[:])
            score_sb = med.tile([P, L], F32, name="score", tag="score")
            ssk = small.tile([1, 2], F32, name="ssk", tag="ssk")
            score_stats(actt, ssk[0:1, 0:2], score_sb[:])
            sin, sout = st2[k]
            nc.sync.dma_start(out=sin[0, 0:2], in_=ssk[0, 0:2])
            if DEBUG_NO_COLLECTIVE:
                nc.sync.dma_start(out=sout[:], in_=sin[:])
            else:
                nc.gpsimd.collective_compute(
                    "AllReduce", ALU.add, replica_groups=[list(range(NCORES))],
                    ins=[sin[:].opt()], outs=[sout[:].opt()])
            sums2 = small.tile([P, 2], F32, name="sumsbc", tag="sumsbc")
            nc.sync.dma_start(out=sums2[:], in_=_bcast_ap(sout[0, 0:2]))
            sc_scale = small.tile([P, 1], F32, name="scsc", tag="scsc")
            sc_shift = small.tile([P, 1], F32, name="scsh", tag="scsh")
            bn_coeffs(sums2, sc_scale[:], sc_shift[:])

    # ---------------- outputs ----------------
    emit_outputs(supt)


_NC_CACHE = []


def _get_nc():
    if not _NC_CACHE:
        _NC_CACHE.append(_build_kernel())
    return _NC_CACHE[0]


def _pack_atom(atom_FP, atom_mask):
    """[B,L,FP] f32 + [B,L] mask -> per-core [L, BLOC, RW] bf16 blocks."""
    at8 = np.asarray(atom_FP, np.float32).reshape(NCORES, BLOC, L, FP)
    m8 = np.asarray(atom_mask, np.float32).reshape(NCORES, BLOC, L)
    pk = np.empty((NCORES, L, BLOC, RW), dtype=ml_dtypes.bfloat16)
    pk[..., :FP] = at8.transpose(0, 2, 1, 3)
    pk[..., FP] = 1.0
    pk[..., FP + 1] = m8.transpose(0, 2, 1)
    return pk


def kernel(atom_FP, atom_mask, align_W, align_b, align_gamma, align_beta,
           attend_W, attend_b, attend_gamma, attend_beta,
           gru_Wih, gru_Whh, gru_bih, gru_bhh, trace=False, **trace_kwargs):
    nc = _get_nc()
    pk = _pack_atom(atom_FP, atom_mask)
    atom_mask = np.ascontiguousarray(np.asarray(atom_mask, dtype=np.float32))
    shared = {
        "align_w": np.asarray(align_W, np.float32),
        "align_b": np.asarray(align_b, np.float32),
        "align_gamma": np.asarray(align_gamma, np.float32),
        "align_beta": np.asarray(align_beta, np.float32),
        "attend_w": np.asarray(attend_W, np.float32),
        "attend_b": np.asarray(attend_b, np.float32),
        "attend_gamma": np.asarray(attend_gamma, np.float32),
        "attend_beta": np.asarray(attend_beta, np.float32),
        "gru_wih": np.asarray(gru_Wih, np.float32),
        "gru_whh": np.asarray(gru_Whh, np.float32),
        "gru_bih": np.asarray(gru_bih, np.float32),
        "gru_bhh": np.asarray(gru_bhh, np.float32),
    }
    in_maps = []
    for c in range(NCORES):
        m = dict(shared)
        m["atom_pk"] = pk[c]
        m["atom_mask"] = atom_mask[c * BLOC:(c + 1) * BLOC]
        in_maps.append(m)
    res = run_bass_kernel_spmd(nc, in_maps, core_ids=list(range(NCORES)),
                               trace=trace, **trace_kwargs)
    sup = np.concatenate([res.results[c]["out_super"] for c in range(NCORES)], 0)
    act = np.concatenate([res.results[c]["out_act"] for c in range(NCORES)], 0)
    if trace:
        kernel.last_exec_time_ns = res.exec_time_ns
        kernel.last_results = res
    return sup, act


# revision 35
# speedup vs baseline: 1.1098x; 1.1098x over previous
"""AttentiveFP molecular readout kernel for 8x Trainium2 NeuronCores.

Data-parallel over the batch (128 molecules/core). Never materializes
xt = atom_FP @ attend_W.T: BatchNorm over xt is affine per channel, so
  sum_l attn*BN(xt) = (sum_l attn*atom) @ (diag(a)W).T + d
with a = rsqrt(var+eps)*gamma and d = a*(b - mean) + beta (sum_l attn
is exactly 1: masked scores underflow to exp(-9e8)=0).  Mean/var of xt
come from the Gram matrix atom^T@atom (one-time PE pass).  Exact BN
batch-stat parity across cores via small AllReduces (one early for the
Gram stats, one per layer for the score stats, plus a warm-up).

Atom data is packed host-side as bf16 [l, b, atom|1|mask] so one HBM
read serves the Gram pass, s2, super0 and every layer's weighted
reduction u = sum_l attn*atom (PE matmuls with the per-molecule atom
block stationary and a 1-column attn vector moving).  Atom DMA rides
the sync queue; all small/gather DMAs are batched onto the scalar
queue so the bulk stream starts immediately.
"""
import os
import sys

sys.path.insert(0, "/opt/trn_rl_repo")

from contextlib import ExitStack

import numpy as np
import ml_dtypes

import concourse.bacc as bacc
import concourse.bass as bass
import concourse.tile as tile
from concourse import masks, mybir
from concourse.bass_utils import run_bass_kernel_spmd

B, L, FP, LAYERS = 1024, 128, 256, 3
NCORES = 8
BLOC = B // NCORES          # 128 molecules per core
N = B * L                   # global BN sample count
EPS = 1e-6
NEG = -900000000.0
F32 = mybir.dt.float32
BF16 = mybir.dt.bfloat16
P = 128
FC = FP // P                # 2 f-chunks of 128
JC = 3 * FP // P            # 6 gate-row chunks of 128
RW = FP + 2                 # packed row: atom | 1.0 | mask
GBLK = 16                   # molecules per DMA chunk
NG = BLOC // GBLK
DEBUG_NO_COLLECTIVE = False
KSTAGE = int(os.environ.get('KSTAGE', '9'))
UTP = int(os.environ.get('UTP', '0'))   # column-tiled u-pass weight loads
ALU = mybir.AluOpType
ACT = mybir.ActivationFunctionType


def _bcast_ap(ap, parts=P):
    """Partition-broadcast view of a 1D/row AP (step-0 partition dim)."""
    return bass.AP(tensor=ap.tensor, offset=ap.offset, ap=[[0, parts]] + list(ap.ap))


def _gather_ap(handle, nchunk, rowlen):
    """Partition-major view of a [nchunk*128, rowlen]-ish flat DRAM tensor:
    dest (p, c) <- src[c*128 + p] (for rowlen==1 vectors) or
    dest (p, c, f) <- src[c*128 + p, f]."""
    base = handle[:]
    if rowlen == 1:
        ap = [[1, P], [P, nchunk]]
    else:
        ap = [[rowlen, P], [P * rowlen, nchunk], [1, rowlen]]
    return bass.AP(tensor=base.tensor, offset=base.offset, ap=ap)


def _build_kernel():
    nc = bacc.Bacc()
    atom_h = nc.declare_dram_parameter("atom_pk", [L, BLOC, RW], BF16, isOutput=False)
    mask_h = nc.declare_dram_parameter("atom_mask", [BLOC, L], F32, isOutput=False)
    alw_h = nc.declare_dram_parameter("align_w", [1, 2 * FP], F32, isOutput=False)
    alb_h = nc.declare_dram_parameter("align_b", [1], F32, isOutput=False)
    alg_h = nc.declare_dram_parameter("align_gamma", [1], F32, isOutput=False)
    albe_h = nc.declare_dram_parameter("align_beta", [1], F32, isOutput=False)
    atw_h = nc.declare_dram_parameter("attend_w", [FP, FP], F32, isOutput=False)
    atb_h = nc.declare_dram_parameter("attend_b", [FP], F32, isOutput=False)
    atg_h = nc.declare_dram_parameter("attend_gamma", [FP], F32, isOutput=False)
    atbe_h = nc.declare_dram_parameter("attend_beta", [FP], F32, isOutput=False)
    wih_h = nc.declare_dram_parameter("gru_wih", [3 * FP, FP], F32, isOutput=False)
    whh_h = nc.declare_dram_parameter("gru_whh", [3 * FP, FP], F32, isOutput=False)
    bih_h = nc.declare_dram_parameter("gru_bih", [3 * FP], F32, isOutput=False)
    bhh_h = nc.declare_dram_parameter("gru_bhh", [3 * FP], F32, isOutput=False)
    osuper_h = nc.declare_dram_parameter("out_super", [BLOC, FP], F32, isOutput=True)
    oact_h = nc.declare_dram_parameter("out_act", [BLOC, FP], F32, isOutput=True)

    with tile.TileContext(nc) as tc, ExitStack() as ctx:
        _body(ctx, tc, atom_h, mask_h, alw_h, alb_h, alg_h, albe_h,
              atw_h, atb_h, atg_h, atbe_h, wih_h, whh_h, bih_h, bhh_h,
              osuper_h, oact_h)
    nc.finalize()
    return nc


def _body(ctx, tc, atom_h, mask_h, alw_h, alb_h, alg_h, albe_h,
          atw_h, atb_h, atg_h, atbe_h, wih_h, whh_h, bih_h, bhh_h,
          osuper_h, oact_h):
    nc = tc.nc
    singles = ctx.enter_context(tc.tile_pool(name="singles", bufs=1))
    small = ctx.enter_context(tc.tile_pool(name="small", bufs=2))
    med = ctx.enter_context(tc.tile_pool(name="med", bufs=2))
    s2p = ctx.enter_context(tc.tile_pool(name="s2p", bufs=2))
    dram = ctx.enter_context(tc.tile_pool(name="dram", bufs=1, space="DRAM"))
    ps_t = ctx.enter_context(tc.tile_pool(name="ps_t", bufs=2, space="PSUM"))
    ps_mm = ctx.enter_context(tc.tile_pool(name="ps_mm", bufs=1, space="PSUM"))

    # ---- atom bulk first on the sync DMA queue --------------------------
    atile = [singles.tile([P, GBLK, RW], BF16, name=f"at{g}", tag=f"at{g}")
             for g in range(NG)]
    for g in range(NG):
        nc.sync.dma_start(out=atile[g][:], in_=atom_h[:, g * GBLK:(g + 1) * GBLK, :])

    # ---- batched prep DMAs on the scalar queue --------------------------
    w_sb = singles.tile([P, FC, FP], F32)
    nc.scalar.dma_start(out=w_sb[:], in_=_gather_ap(atw_h, FC, FP))
    gru_ih = singles.tile([P, JC, FP], F32)
    gru_hh = singles.tile([P, JC, FP], F32)
    nc.scalar.dma_start(out=gru_ih[:], in_=_gather_ap(wih_h, JC, FP))
    nc.scalar.dma_start(out=gru_hh[:], in_=_gather_ap(whh_h, JC, FP))
    w1_pm = singles.tile([P, FC], F32)
    nc.scalar.dma_start(out=w1_pm[:], in_=_gather_ap(alw_h, FC, 1))
    atb_pm = singles.tile([P, FC], F32)
    atg_pm = singles.tile([P, FC], F32)
    atbe_pm = singles.tile([P, FC], F32)
    nc.scalar.dma_start(out=atb_pm[:], in_=_gather_ap(atb_h, FC, 1))
    nc.scalar.dma_start(out=atg_pm[:], in_=_gather_ap(atg_h, FC, 1))
    nc.scalar.dma_start(out=atbe_pm[:], in_=_gather_ap(atbe_h, FC, 1))
    bih_pm = singles.tile([P, JC], F32)
    bhh_pm = singles.tile([P, JC], F32)
    nc.scalar.dma_start(out=bih_pm[:], in_=_gather_ap(bih_h, JC, 1))
    nc.scalar.dma_start(out=bhh_pm[:], in_=_gather_ap(bhh_h, JC, 1))
    w2_bc = singles.tile([P, FP], F32)
    nc.scalar.dma_start(out=w2_bc[:], in_=_bcast_ap(alw_h[0, FP:2 * FP]))
    al_bc = singles.tile([P, 3], F32)   # [align_b, align_gamma, align_beta]
    nc.scalar.dma_start(out=al_bc[:, 0], in_=_bcast_ap(alb_h[0:1])[:, 0])
    nc.scalar.dma_start(out=al_bc[:, 1], in_=_bcast_ap(alg_h[0:1])[:, 0])
    nc.scalar.dma_start(out=al_bc[:, 2], in_=_bcast_ap(albe_h[0:1])[:, 0])
    mask_sb = singles.tile([P, L], F32)
    nc.scalar.dma_start(out=mask_sb[:], in_=mask_h[:, :])

    # ---- constants ------------------------------------------------------
    ident = singles.tile([P, P], F32)
    masks.make_identity(nc, ident[:])
    ident_bf = singles.tile([P, P], BF16)
    nc.vector.tensor_copy(ident_bf[:], ident[:])
    zero_pm = singles.tile([P, 1], F32)
    nc.vector.memset(zero_pm[:], 0.0)
    eps_pm = singles.tile([P, 1], F32)
    nc.vector.memset(eps_pm[:], EPS)
    ones_pm = singles.tile([P, 1], F32)
    nc.vector.memset(ones_pm[:], 1.0)

    def pe_t(out_sb, in_sb):
        """128x128 fp32 transpose via PE; out_sb <- in_sb.T"""
        pt = ps_t.tile([P, P], F32, name="ps_t", tag="ps_t")
        nc.tensor.transpose(pt[:], in_sb, ident[:])
        nc.vector.tensor_copy(out_sb, pt[:])

    # w2 replicated across the packed-chunk layout (zeros on ones/mask cols)
    w2b_bf = singles.tile([P, FP], BF16)
    nc.vector.tensor_copy(w2b_bf[:], w2_bc[:])
    w2rep = singles.tile([P, GBLK, RW], BF16)
    nc.vector.memset(w2rep[:], 0.0)
    for j in range(GBLK):
        nc.vector.tensor_copy(w2rep[:, j, 0:FP], w2b_bf[:])

    # attend-W transposes (needed early for dgw); GRU transposes deferred
    wt_sb = [singles.tile([P, FP], F32, name=f"wt_sb{i}", tag=f"wt_sb{i}") for i in range(FC)]
    for ic in range(FC):
        for fc in range(FC):
            pe_t(wt_sb[ic][:, fc * P:(fc + 1) * P],
                 w_sb[:, fc, ic * P:(ic + 1) * P])
    negm_sb = singles.tile([P, L], F32)
    nc.vector.tensor_scalar(out=negm_sb[:], in0=mask_sb[:], scalar1=1.0,
                            scalar2=-NEG, op0=ALU.subtract, op1=ALU.mult)
    bsum_pm = singles.tile([P, 4], F32)
    nc.vector.tensor_tensor(out=bsum_pm[:], in0=bih_pm[:, 0:4], in1=bhh_pm[:, 0:4],
                            op=ALU.add)

    if KSTAGE <= 0:
        dm = med.tile([P, FP], F32, name="dm", tag="dm", bufs=1)
        nc.vector.tensor_copy(dm[:], w_sb[:, 0, :])
        nc.sync.dma_start(out=osuper_h[:, :], in_=dm[:])
        nc.sync.dma_start(out=oact_h[:, :], in_=gru_ih[:, 0, :])
        return

    # ---------------- pass 1: Gram + supt (PE), s2 (DVE) ----------------
    gram_pool = tc.tile_pool(name="ps_gram", bufs=1, space="PSUM")
    ps_gram = gram_pool.__enter__()
    gram_ps = [ps_gram.tile([P, FP + 1], F32, name=f"gram{i}", tag=f"gram{i}") for i in range(FC)]
    supt_ps = ps_gram.tile([P, FC, P], F32, name="supt_ps", tag="supt_ps")
    s2t_sb = singles.tile([P, BLOC], F32)
    for g in range(NG):
        at = atile[g]
        for j in range(GBLK):
            b = g * GBLK + j
            for fc in range(FC):
                wsl = at[:, j, fc * P:(fc + 1) * P]
                nc.tensor.matmul(gram_ps[fc][:], wsl, at[:, j, 0:FP + 1],
                                 start=(b == 0), stop=(b == BLOC - 1))
                nc.tensor.matmul(supt_ps[:, fc, b:b + 1], wsl,
                                 at[:, j, FP + 1:FP + 2], start=True, stop=True)
        # s2 for this chunk: one dense product then per-molecule reduce.
        # The last 3 chunks are deferred past the AllReduce-1a trigger so
        # the Gram copies/dgw don't queue behind them on the DVE.
        if g < NG - 3:
            prod = s2p.tile([P, GBLK, RW], BF16, name="s2prod", tag="s2prod")
            nc.vector.tensor_tensor(out=prod[:], in0=at[:], in1=w2rep[:], op=ALU.mult)
            nc.vector.tensor_reduce(out=s2t_sb[:, g * GBLK:(g + 1) * GBLK],
                                    in_=prod[:], axis=mybir.AxisListType.X, op=ALU.add)

    g_sb = [singles.tile([P, FP + 1], F32, name=f"g_sb{i}", tag=f"g_sb{i}") for i in range(FC)]
    supt = [med.tile([P, P], F32, name=f"supt{i}", tag=f"supt{i}") for i in range(FC)]
    for fc in range(FC):
        nc.vector.tensor_copy(g_sb[fc][:], gram_ps[fc][:])
        nc.vector.tensor_copy(supt[fc][:], supt_ps[:, fc, :])
    gram_pool.__exit__(None, None, None)
    ps_g = ctx.enter_context(tc.tile_pool(name="ps_g", bufs=2, space="PSUM"))
    ps_u = ctx.enter_context(tc.tile_pool(name="ps_u", bufs=1, space="PSUM"))
    s2_sb = singles.tile([P, L], F32)

    def emit_outputs(supt_tiles):
        super_sb = med.tile([P, FP], F32, name="souts", tag="souts", bufs=1)
        for ic in range(FC):
            pe_t(super_sb[:, ic * P:(ic + 1) * P], supt_tiles[ic][:])
        act_sb = med.tile([P, FP], F32, name="souta", tag="souta", bufs=1)
        nc.scalar.activation(act_sb[:], super_sb[:], ACT.Relu, bias=zero_pm[:])
        nc.sync.dma_start(out=osuper_h[:, :], in_=super_sb[:])
        nc.sync.dma_start(out=oact_h[:, :], in_=act_sb[:])

    if KSTAGE <= 1:
        emit_outputs(supt)
        return

    # diag(W G W^T) per f-chunk -> [128,1]
    tt_scr = small.tile([P, FP], F32, name="ttscr", tag="ttscr")
    dgw_pm = [small.tile([P, 1], F32, name=f"dgw{i}", tag=f"dgw{i}") for i in range(FC)]
    for fc in range(FC):
        t1 = ps_mm.tile([P, FP], F32, name="t1", tag="mm")
        for ic in range(FC):
            nc.tensor.matmul(t1[:], wt_sb[ic][:, fc * P:(fc + 1) * P],
                             g_sb[ic][:, 0:FP],
                             start=(ic == 0), stop=(ic == FC - 1))
        nc.vector.tensor_tensor(out=tt_scr[:], in0=t1[:], in1=w_sb[:, fc, :],
                                op=ALU.mult)
        nc.vector.tensor_reduce(out=dgw_pm[fc][:], in_=tt_scr[:],
                                axis=mybir.AxisListType.X, op=ALU.add)

    # ---------------- AllReduce #1a: Gram-derived stats -----------------
    st_in = dram.tile([1, 2 * FP], F32, name="st_in", tag="st_in")
    st_out = dram.tile([1, 2 * FP], F32, name="st_out", tag="st_out")
    for fc in range(FC):
        nc.sync.dma_start(out=st_in[0, fc * P:(fc + 1) * P], in_=dgw_pm[fc][:, 0])
        nc.sync.dma_start(out=st_in[0, FP + fc * P:FP + (fc + 1) * P],
                          in_=g_sb[fc][:, FP])
    if DEBUG_NO_COLLECTIVE:
        nc.sync.dma_start(out=st_out[:], in_=st_in[:])
    else:
        nc.gpsimd.collective_compute(
            "AllReduce", ALU.add, replica_groups=[list(range(NCORES))],
            ins=[st_in[:].opt()], outs=[st_out[:].opt()])
    dgw_g = [small.tile([P, 1], F32, name=f"dgwg{i}", tag=f"dgwg{i}") for i in range(FC)]
    cs_g = [small.tile([P, 1], F32, name=f"csg{i}", tag=f"csg{i}") for i in range(FC)]
    for fc in range(FC):
        nc.sync.dma_start(out=dgw_g[fc][:, 0], in_=st_out[0, fc * P:(fc + 1) * P])
        nc.sync.dma_start(out=cs_g[fc][:, 0],
                          in_=st_out[0, FP + fc * P:FP + (fc + 1) * P])

    # GRU weight transposes: fill the PE queue while AllReduce 1a flies
    wiht = [singles.tile([P, 3 * FP], F32, name=f"wiht{i}", tag=f"wiht{i}") for i in range(FC)]
    whht = [singles.tile([P, 3 * FP], F32, name=f"whht{i}", tag=f"whht{i}") for i in range(FC)]
    for src, dst in ((gru_ih, wiht), (gru_hh, whht)):
        for jc in range(JC):
            for ic in range(FC):
                pe_t(dst[ic][:, jc * P:(jc + 1) * P],
                     src[:, jc, ic * P:(ic + 1) * P])

    # deferred s2 chunks run while AllReduce #1a is in flight
    for g in range(NG - 3, NG):
        prod = s2p.tile([P, GBLK, RW], BF16, name="s2prod", tag="s2prod")
        nc.vector.tensor_tensor(out=prod[:], in0=atile[g][:], in1=w2rep[:], op=ALU.mult)
        nc.vector.tensor_reduce(out=s2t_sb[:, g * GBLK:(g + 1) * GBLK],
                                in_=prod[:], axis=mybir.AxisListType.X, op=ALU.add)
    pe_t(s2_sb[:], s2t_sb[:])

    # ---------------- layer-0 state + score stats -----------------------
    actt = [med.tile([P, P], F32, name=f"actt{i}", tag=f"actt{i}") for i in range(FC)]
    for ic in range(FC):
        nc.scalar.activation(actt[ic][:], supt[ic][:], ACT.Relu, bias=zero_pm[:])

    def score_stats(actt_tiles, ss2_sb, score_sb):
        """score = s1 + s2 + align_b; writes local sum/sumsq scalars [1,2]."""
        s1_ps = ps_mm.tile([P, 1], F32, name="s1", tag="mm")
        for ic in range(FC):
            nc.tensor.matmul(s1_ps[:], actt_tiles[ic][:], w1_pm[:, ic:ic + 1],
                             start=(ic == 0), stop=(ic == FC - 1))
        s1_sb = small.tile([P, 1], F32, name="s1sb", tag="s1sb")
        nc.vector.tensor_copy(s1_sb[:], s1_ps[:])
        nc.vector.tensor_scalar(out=score_sb, in0=s2_sb[:], scalar1=s1_sb[:],
                                scalar2=al_bc[:, 0:1], op0=ALU.add, op1=ALU.add)
        red2 = small.tile([P, 2], F32, name="red2", tag="red2")
        nc.vector.tensor_reduce(out=red2[:, 0:1], in_=score_sb,
                                axis=mybir.AxisListType.X, op=ALU.add)
        scr = small.tile([P, L], F32, name="sqscr", tag="sqscr")
        nc.vector.tensor_tensor(out=scr[:], in0=score_sb, in1=score_sb,
                                op=ALU.mult)
        nc.vector.tensor_reduce(out=red2[:, 1:2], in_=scr[:],
                                axis=mybir.AxisListType.X, op=ALU.add)
        pred = ps_mm.tile([1, 2], F32, name="pred", tag="mm")
        nc.tensor.matmul(pred[:], ones_pm[:], red2[:], start=True, stop=True)
        nc.vector.tensor_copy(ss2_sb, pred[:])

    score_sb0 = med.tile([P, L], F32, name="score", tag="score")
    ss_sb = small.tile([1, 2], F32, name="ss0", tag="ss0")
    score_stats(actt, ss_sb[0:1, 0:2], score_sb0[:])

    # ---------------- AllReduce #1b: layer-0 score stats ----------------
    st1b_in = dram.tile([1, 2], F32, name="st1bi", tag="st1bi")
    st1b_out = dram.tile([1, 2], F32, name="st1bo", tag="st1bo")
    nc.sync.dma_start(out=st1b_in[0, 0:2], in_=ss_sb[0, 0:2])
    if DEBUG_NO_COLLECTIVE:
        nc.sync.dma_start(out=st1b_out[:], in_=st1b_in[:])
    else:
        nc.gpsimd.collective_compute(
            "AllReduce", ALU.add, replica_groups=[list(range(NCORES))],
            ins=[st1b_in[:].opt()], outs=[st1b_out[:].opt()])
    sums_bc = small.tile([P, 2], F32, name="sumsbc", tag="sumsbc")
    nc.sync.dma_start(out=sums_bc[:], in_=_bcast_ap(st1b_out[0, 0:2]))

    if KSTAGE <= 2:
        emit_outputs(supt)
        return

    # xt-BN affine: a = rsqrt(v+eps)*gamma, d = a*(b-m)+beta  (partition-major)
    a_pm = [small.tile([P, 1], F32, name=f"apm{i}", tag=f"apm{i}") for i in range(FC)]
    d_pm = [small.tile([P, 1], F32, name=f"dpm{i}", tag=f"dpm{i}") for i in range(FC)]
    adr = dram.tile([1, 2 * FP], F32, name="adr", tag="adr")
    for fc in range(FC):
        wc = ps_mm.tile([P, 1], F32, name="wc", tag="mm")
        for ic in range(FC):
            nc.tensor.matmul(wc[:], wt_sb[ic][:, fc * P:(fc + 1) * P], cs_g[ic][:],
                             start=(ic == 0), stop=(ic == FC - 1))
        m_xt = small.tile([P, 1], F32, name="mxt", tag="mxt")
        nc.vector.tensor_scalar(out=m_xt[:], in0=wc[:], scalar1=1.0 / N,
                                scalar2=atb_pm[:, fc:fc + 1], op0=ALU.mult, op1=ALU.add)
        e2 = small.tile([P, 1], F32, name="e2", tag="e2")
        # e2 = dgw/N + 2*b*wc/N + b^2
        nc.vector.tensor_scalar(out=e2[:], in0=wc[:], scalar1=2.0 / N,
                                scalar2=atb_pm[:, fc:fc + 1], op0=ALU.mult, op1=ALU.mult)
        nc.vector.tensor_scalar(out=tt_scr[:, 0:1], in0=dgw_g[fc][:],
                                scalar1=1.0 / N, scalar2=None, op0=ALU.mult)
        nc.vector.tensor_add(out=e2[:], in0=e2[:], in1=tt_scr[:, 0:1])
        nc.vector.tensor_tensor(out=tt_scr[:, 1:2], in0=atb_pm[:, fc:fc + 1],
                                in1=atb_pm[:, fc:fc + 1], op=ALU.mult)
        nc.vector.tensor_add(out=e2[:], in0=e2[:], in1=tt_scr[:, 1:2])
        # v = e2 - m^2 ; a = gamma/sqrt(v+eps)
        nc.vector.tensor_tensor(out=tt_scr[:, 2:3], in0=m_xt[:], in1=m_xt[:],
                                op=ALU.mult)
        nc.vector.tensor_sub(out=e2[:], in0=e2[:], in1=tt_scr[:, 2:3])
        nc.scalar.activation(e2[:], e2[:], ACT.Sqrt, bias=eps_pm[:])
        nc.vector.reciprocal(e2[:], e2[:])
        nc.vector.tensor_tensor(out=a_pm[fc][:], in0=e2[:], in1=atg_pm[:, fc:fc + 1],
                                op=ALU.mult)
        # d = a*(b - m) + beta
        nc.vector.tensor_sub(out=tt_scr[:, 3:4], in0=atb_pm[:, fc:fc + 1], in1=m_xt[:])
        nc.vector.tensor_tensor(out=tt_scr[:, 3:4], in0=tt_scr[:, 3:4],
                                in1=a_pm[fc][:], op=ALU.mult)
        nc.vector.tensor_scalar(out=d_pm[fc][:], in0=tt_scr[:, 3:4], scalar1=0.0,
                                scalar2=atbe_pm[:, fc:fc + 1], op0=ALU.add, op1=ALU.add)
        nc.sync.dma_start(out=adr[0, fc * P:(fc + 1) * P], in_=a_pm[fc][:, 0])
        nc.sync.dma_start(out=adr[0, FP + fc * P:FP + (fc + 1) * P],
                          in_=d_pm[fc][:, 0])
    a_bc = singles.tile([P, FP], F32)
    d_bc = singles.tile([P, FP], F32)
    nc.sync.dma_start(out=a_bc[:], in_=_bcast_ap(adr[0, 0:FP]))
    nc.sync.dma_start(out=d_bc[:], in_=_bcast_ap(adr[0, FP:2 * FP]))
    wpt = [singles.tile([P, FP], F32, name=f"wpt{i}", tag=f"wpt{i}") for i in range(FC)]
    for ic in range(FC):
        nc.vector.tensor_tensor(out=wpt[ic][:], in0=wt_sb[ic][:], in1=a_bc[:],
                                op=ALU.mult)

    def bn_coeffs(sums_bc_ap, scale_sb, shift_sb):
        """score-BN: scale = gamma*rstd, shift = beta - mean*scale (per-part)."""
        mean = small.tile([P, 1], F32, name="bnm", tag="bnm")
        nc.vector.tensor_scalar(out=mean[:], in0=sums_bc_ap[:, 0:1],
                                scalar1=1.0 / N, scalar2=None, op0=ALU.mult)
        var = small.tile([P, 1], F32, name="bnv", tag="bnv")
        nc.vector.tensor_scalar(out=var[:], in0=sums_bc_ap[:, 1:2],
                                scalar1=1.0 / N, scalar2=None, op0=ALU.mult)
        nc.vector.tensor_tensor(out=scale_sb, in0=mean[:], in1=mean[:],
                                op=ALU.mult)
        nc.vector.tensor_sub(out=var[:], in0=var[:], in1=scale_sb)
        nc.scalar.activation(var[:], var[:], ACT.Sqrt, bias=eps_pm[:])
        nc.vector.reciprocal(var[:], var[:])
        nc.vector.tensor_tensor(out=scale_sb, in0=var[:], in1=al_bc[:, 1:2],
                                op=ALU.mult)
        nc.vector.tensor_tensor(out=shift_sb, in0=mean[:], in1=scale_sb,
                                op=ALU.mult)
        nc.vector.tensor_scalar(out=shift_sb, in0=shift_sb, scalar1=-1.0,
                                scalar2=al_bc[:, 2:3], op0=ALU.mult, op1=ALU.add)

    sc_scale = small.tile([P, 1], F32, name="scsc", tag="scsc")
    sc_shift = small.tile([P, 1], F32, name="scsh", tag="scsh")
    bn_coeffs(sums_bc, sc_scale[:], sc_shift[:])

    if KSTAGE <= 3:
        emit_outputs(supt)
        return

    # ---------------- layers ----------------
    score_sb = score_sb0
    st2 = [(dram.tile([1, 2], F32, name=f"st2i{k}", tag=f"st2i{k}"),
            dram.tile([1, 2], F32, name=f"st2o{k}", tag=f"st2o{k}")) for k in range(2)]

    nlayers = 1 if KSTAGE <= 4 else LAYERS
    for k in range(nlayers):
        # BN + leaky-relu (fused) + additive mask
        y_sb = med.tile([P, L], F32, name="y", tag="y")
        nc.vector.tensor_scalar(out=y_sb[:], in0=score_sb[:], scalar1=sc_scale[:],
                                scalar2=sc_shift[:], op0=ALU.mult, op1=ALU.add)
        lr_r = med.tile([P, L], F32, name="lr_r", tag="lr_r")
        nc.scalar.activation(lr_r[:], y_sb[:], ACT.Relu, bias=zero_pm[:])
        nc.vector.tensor_sub(out=y_sb[:], in0=y_sb[:], in1=lr_r[:])
        nc.vector.tensor_scalar(out=y_sb[:], in0=y_sb[:], scalar1=0.01,
                                scalar2=None, op0=ALU.mult)
        nc.vector.tensor_add(out=y_sb[:], in0=y_sb[:], in1=lr_r[:])
        nc.vector.tensor_add(out=y_sb[:], in0=y_sb[:], in1=negm_sb[:])
        # softmax over l
        rmax = small.tile([P, 1], F32, name="rmax", tag="rmax")
        nc.vector.tensor_reduce(out=rmax[:], in_=y_sb[:],
                                axis=mybir.AxisListType.X, op=ALU.max,
                                negate=True)
        e_sb = med.tile([P, L], F32, name="esb", tag="esb")
        den = small.tile([P, 1], F32, name="den", tag="den")
        nc.vector.tensor_scalar(out=e_sb[:], in0=y_sb[:], scalar1=rmax[:],
                                scalar2=-80.0, op0=ALU.add, op1=ALU.max)
        nc.scalar.activation(e_sb[:], e_sb[:], ACT.Exp, bias=zero_pm[:])
        nc.vector.tensor_reduce(out=den[:], in_=e_sb[:],
                                axis=mybir.AxisListType.X, op=ALU.add)
        nc.vector.reciprocal(den[:], den[:])
        # attn (bf16, transposed to [l, b]); mask applied via packed mask col
        at_bf = med.tile([P, L], BF16, name="atbf", tag="atbf")
        nc.vector.tensor_scalar(out=at_bf[:], in0=e_sb[:], scalar1=den[:],
                                scalar2=None, op0=ALU.mult)
        att_ps = ps_t.tile([P, P], BF16, name="attps", tag="ps_t")
        nc.tensor.matmul(att_ps[:], at_bf[:], ident_bf[:], is_transpose=True)
        attnT = med.tile([P, P], BF16, name="attnT", tag="attnT")
        for g in range(NG):
            nc.vector.tensor_tensor(
                out=attnT[:, g * GBLK:(g + 1) * GBLK],
                in0=att_ps[:, g * GBLK:(g + 1) * GBLK],
                in1=atile[g][:, :, FP + 1], op=ALU.mult)

        # u^T = sum_l attn*atom via PE: atom block stationary, attn col moving
        u_ps = ps_u.tile([P, FC, P], F32, name="u_ps", tag="u_ps")
        for g in range(NG):
            at = atile[g]
            for j in range(GBLK):
                b = g * GBLK + j
                for fc in range(FC):
                    if UTP:
                        for t in range(4):
                            c0 = fc * P + 32 * t
                            nc.tensor.matmul(
                                u_ps[32 * t:32 * (t + 1), fc, b:b + 1],
                                at[:, j, c0:c0 + 32], attnT[:, b:b + 1],
                                start=True, stop=True, tile_position=(0, 32 * t))
                    else:
                        nc.tensor.matmul(u_ps[:, fc, b:b + 1],
                                         at[:, j, fc * P:(fc + 1) * P],
                                         attnT[:, b:b + 1], start=True, stop=True)
        u_sb = [med.tile([P, P], F32, name=f"usb{i}", tag=f"usb{i}") for i in range(FC)]
        for fc in range(FC):
            nc.vector.tensor_copy(u_sb[fc][:], u_ps[:, fc, :])

        # ctx = elu(u @ Wp.T + d)   (sum_l attn == 1 exactly)
        cpre_ps = ps_mm.tile([P, FP], F32, name="cpre", tag="mm")
        for ic in range(FC):
            nc.tensor.matmul(cpre_ps[:], u_sb[ic][:], wpt[ic][:],
                             start=(ic == 0), stop=(ic == FC - 1))
        cpre = med.tile([P, FP], F32, name="cprs", tag="cprs", bufs=1)
        nc.vector.tensor_add(out=cpre[:], in0=cpre_ps[:], in1=d_bc[:])
        relu_p = med.tile([P, FP], F32, name="relup", tag="relup", bufs=1)
        nc.scalar.activation(relu_p[:], cpre[:], ACT.Relu, bias=zero_pm[:])
        nc.vector.tensor_sub(out=cpre[:], in0=cpre[:], in1=relu_p[:])
        nc.scalar.activation(cpre[:], cpre[:], ACT.Exp, bias=zero_pm[:])
        ctx_sb = med.tile([P, FP], F32, name="ctx", tag="ctx", bufs=1)
        nc.vector.tensor_scalar(out=ctx_sb[:], in0=cpre[:], scalar1=1.0,
                                scalar2=None, op0=ALU.subtract)
        nc.vector.tensor_add(out=ctx_sb[:], in0=ctx_sb[:], in1=relu_p[:])

        # GRU (transposed form)
        ctxt = [med.tile([P, P], F32, name=f"ctxt{i}", tag=f"ctxt{i}") for i in range(FC)]
        for ic in range(FC):
            pe_t(ctxt[ic][:], ctx_sb[:, ic * P:(ic + 1) * P])
        new_supt = [med.tile([P, P], F32, name=f"nsupt{i}", tag=f"nsupt{i}")
                    for i in range(FC)]
        for c in range(FC):
            t = med.tile([P, P], F32, name="gt", tag="gt")
            r = med.tile([P, P], F32, name="gr", tag="gr")
            z = med.tile([P, P], F32, name="gz", tag="gz")
            nn_ = med.tile([P, P], F32, name="gn", tag="gn")
            grz = ps_g.tile([P, P], F32, name="grz", tag="grz")
            for ic in range(FC):
                nc.tensor.matmul(grz[:], wiht[ic][:, c * P:(c + 1) * P],
                                 ctxt[ic][:], start=(ic == 0), stop=False)
            for ic in range(FC):
                nc.tensor.matmul(grz[:], whht[ic][:, c * P:(c + 1) * P],
                                 supt[ic][:], start=False, stop=(ic == FC - 1))
            nc.scalar.activation(r[:], grz[:], ACT.Sigmoid, bias=bsum_pm[:, c:c + 1])
            grz2 = ps_g.tile([P, P], F32, name="grz2", tag="grz")
            jz = 2 + c
            for ic in range(FC):
                nc.tensor.matmul(grz2[:], wiht[ic][:, jz * P:(jz + 1) * P],
                                 ctxt[ic][:], start=(ic == 0), stop=False)
            for ic in range(FC):
                nc.tensor.matmul(grz2[:], whht[ic][:, jz * P:(jz + 1) * P],
                                 supt[ic][:], start=False, stop=(ic == FC - 1))
            nc.scalar.activation(z[:], grz2[:], ACT.Sigmoid, bias=bsum_pm[:, jz:jz + 1])
            jn = 4 + c
            gin = ps_g.tile([P, P], F32, name="gin", tag="gin", bufs=1)
            ghn = ps_g.tile([P, P], F32, name="ghn", tag="ghn", bufs=1)
            for ic in range(FC):
                nc.tensor.matmul(gin[:], wiht[ic][:, jn * P:(jn + 1) * P],
                                 ctxt[ic][:], start=(ic == 0), stop=(ic == FC - 1))
                nc.tensor.matmul(ghn[:], whht[ic][:, jn * P:(jn + 1) * P],
                                 supt[ic][:], start=(ic == 0), stop=(ic == FC - 1))
            nc.vector.tensor_scalar(out=t[:], in0=ghn[:],
                                    scalar1=bhh_pm[:, jn:jn + 1], scalar2=None,
                                    op0=ALU.add)
            nc.vector.tensor_tensor(out=t[:], in0=t[:], in1=r[:], op=ALU.mult)
            nc.vector.tensor_add(out=t[:], in0=t[:], in1=gin[:])
            nc.scalar.activation(nn_[:], t[:], ACT.Tanh, bias=bih_pm[:, jn:jn + 1])
            # super = n + z*(super - n)
            nc.vector.tensor_sub(out=t[:], in0=supt[c][:], in1=nn_[:])
            nc.vector.tensor_tensor(out=t[:], in0=t[:], in1=z[:], op=ALU.mult)
            nc.vector.tensor_add(out=new_supt[c][:], in0=nn_[:], in1=t[:])
        supt = new_supt

        if KSTAGE <= 4 and k == 0:
            emit_outputs(supt)
            return
        if k < nlayers - 1:
            actt = [med.tile([P, P], F32, name=f"actt{i}", tag=f"actt{i}") for i in range(FC)]
            for ic in range(FC):
                nc.scalar.activation(actt[ic][:], supt[ic][:], ACT.Relu, bias=zero_pm[:])
            score_sb = med.tile([P, L], F32, name="score", tag="score")
            ssk = small.tile([1, 2], F32, name="ssk", tag="ssk")
            score_stats(actt, ssk[0:1, 0:2], score_sb[:])
            sin, sout = st2[k]
            nc.sync.dma_start(out=sin[0, 0:2], in_=ssk[0, 0:2])
            if DEBUG_NO_COLLECTIVE:
                nc.sync.dma_start(out=sout[:], in_=sin[:])
            else:
                nc.gpsimd.collective_compute(
                    "AllReduce", ALU.add, replica_groups=[list(range(NCORES))],
                    ins=[sin[:].opt()], outs=[sout[:].opt()])
            sums2 = small.tile([P, 2], F32, name="sumsbc", tag="sumsbc")
            nc.sync.dma_start(out=sums2[:], in_=_bcast_ap(sout[0, 0:2]))
            sc_scale = small.tile([P, 1], F32, name="scsc", tag="scsc")
            sc_shift = small.tile([P, 1], F32, name="scsh", tag="scsh")
            bn_coeffs(sums2, sc_scale[:], sc_shift[:])

    # ---------------- outputs ----------------
    emit_outputs(supt)


_NC_CACHE = []


def _get_nc():
    if not _NC_CACHE:
        _NC_CACHE.append(_build_kernel())
    return _NC_CACHE[0]


def _pack_atom(atom_FP, atom_mask):
    """[B,L,FP] f32 + [B,L] mask -> per-core [L, BLOC, RW] bf16 blocks."""
    at8 = np.asarray(atom_FP, np.float32).reshape(NCORES, BLOC, L, FP)
    m8 = np.asarray(atom_mask, np.float32).reshape(NCORES, BLOC, L)
    pk = np.empty((NCORES, L, BLOC, RW), dtype=ml_dtypes.bfloat16)
    pk[..., :FP] = at8.transpose(0, 2, 1, 3)
    pk[..., FP] = 1.0
    pk[..., FP + 1] = m8.transpose(0, 2, 1)
    return pk


def kernel(atom_FP, atom_mask, align_W, align_b, align_gamma, align_beta,
           attend_W, attend_b, attend_gamma, attend_beta,
           gru_Wih, gru_Whh, gru_bih, gru_bhh, trace=False, **trace_kwargs):
    nc = _get_nc()
    pk = _pack_atom(atom_FP, atom_mask)
    atom_mask = np.ascontiguousarray(np.asarray(atom_mask, dtype=np.float32))
    shared = {
        "align_w": np.asarray(align_W, np.float32),
        "align_b": np.asarray(align_b, np.float32),
        "align_gamma": np.asarray(align_gamma, np.float32),
        "align_beta": np.asarray(align_beta, np.float32),
        "attend_w": np.asarray(attend_W, np.float32),
        "attend_b": np.asarray(attend_b, np.float32),
        "attend_gamma": np.asarray(attend_gamma, np.float32),
        "attend_beta": np.asarray(attend_beta, np.float32),
        "gru_wih": np.asarray(gru_Wih, np.float32),
        "gru_whh": np.asarray(gru_Whh, np.float32),
        "gru_bih": np.asarray(gru_bih, np.float32),
        "gru_bhh": np.asarray(gru_bhh, np.float32),
    }
    in_maps = []
    for c in range(NCORES):
        m = dict(shared)
        m["atom_pk"] = pk[c]
        m["atom_mask"] = atom_mask[c * BLOC:(c + 1) * BLOC]
        in_maps.append(m)
    res = run_bass_kernel_spmd(nc, in_maps, core_ids=list(range(NCORES)),
                               trace=trace, **trace_kwargs)
    sup = np.concatenate([res.results[c]["out_super"] for c in range(NCORES)], 0)
    act = np.concatenate([res.results[c]["out_act"] for c in range(NCORES)], 0)
    if trace:
        kernel.last_exec_time_ns = res.exec_time_ns
        kernel.last_results = res
    return sup, act
